# revision 1
# baseline (speedup 1.0000x reference)
"""Trainium2 Bass kernel for per-class variance-trace (segment reduction).

Computes, for x[N, D] (fp32) and t[N] (int32 class ids in [0, 10)):
    out = mean_c( sum_d unbiased_var(x[t == c, d]) )

Strategy (8-way data parallel over N):
  Each core gets an equal shard of N rows. Per 128-row subtile the kernel
  builds a one-hot matrix O[128, 10] from t on the vector engine and uses
  the tensor engine to accumulate into PSUM:
      sums[10, 128]  += O.T @ X        (fp16 inputs, fp32 accumulation)
      ssq [10, 128]  += O.T @ X^2      (fp16 inputs, fp32 accumulation)
  The fp16 cast of x comes from the scalar engine (ACT Copy); the squares
  from the vector engine (fp16 x fp16 multiply in 2x mode).
  Counts are accumulated on the vector engine (sum of one-hots per
  partition) and reduced across partitions on the host.
  The tiny per-core partials are summed on the host, and the final
  variance/trace arithmetic happens on the host in float64.

  Uncentered sum-of-squares is numerically safe here: means are ~0 so the
  correction term sums^2/count is ~1e-5 of ssq, which also makes the
  reduced-precision (fp16) matmul inputs harmless to the result. fp16 is
  chosen over bf16 for the extra 3 mantissa bits: the bf16 rounding of
  x^2 introduced a systematic ~1.3e-4 bias in the variance; fp16 brings
  it to the fp32 reference's own noise floor (~2e-5).
"""

import sys

sys.path.insert(0, "/opt/trn_rl_repo")

import numpy as np

NUM_CLASSES = 10
N = 1_000_000
D = 128
P = 128
NCORES = 8
NSHARD = N // NCORES  # 125_000 rows per core

G = 61  # subtiles per group (976 = 16 * 61; 3.9 MB per x DMA)
XBUFS = 3  # x-tile buffer depth (DMA in-flight depth)

_CACHE = {}


def _build(ns, g, xbufs=XBUFS, sqbufs=2):
    """Build + compile the per-core Bass program for a shard of `ns` rows.

    ns = P * qmain + tail with qmain % g == 0 required.
    Returns (nc, main_out_name, cnt_out_name).
    """
    from concourse import bacc, mybir
    import concourse.tile as tile

    f32 = mybir.dt.float32
    f32r = mybir.dt.float32r
    f16 = mybir.dt.float16
    i32 = mybir.dt.int32
    eq = mybir.AluOpType.is_equal
    add = mybir.AluOpType.add
    C = NUM_CLASSES

    qmain = ns // P
    tail = ns - qmain * P
    assert qmain % g == 0, (ns, qmain, g)
    # Group schedule: full-size groups, with the final group tapered into
    # progressively smaller chunks so the last DMA's dependent compute chain
    # (ACT cast -> DVE square -> PE matmuls) is short instead of ~5us.
    groups = []
    pos = 0
    while qmain - pos > g:
        groups.append((pos, g))
        pos += g
    rem = qmain - pos
    while rem > 0:
        take = (rem + 1) // 2 if rem > 2 else rem
        groups.append((pos, take))
        pos += take
        rem -= take
    assert pos == qmain and sum(gl for _, gl in groups) == qmain

    nc = bacc.Bacc("TRN2", target_bir_lowering=False, debug=False)
    x_d = nc.dram_tensor("x", [ns, D], f32, kind="ExternalInput")
    t_d = nc.dram_tensor("t", [ns], i32, kind="ExternalInput")
    out_d = nc.dram_tensor("out", [C, 2 * D], f32, kind="ExternalOutput")
    cnt_d = nc.dram_tensor("cnt", [P, C], f32, kind="ExternalOutput")

    # Row mapping: partition p of subtile q holds DRAM row p*qmain + q, so a
    # group of g subtiles is a contiguous g-row (g*D*4 byte) read per partition.
    x_main = x_d.ap()[0 : qmain * P, :].rearrange("(p q) d -> p q d", p=P)
    t_main = t_d.ap()[0 : qmain * P].rearrange("(p q) -> p q", p=P)

    with tile.TileContext(nc) as tc:
        with (
            tc.tile_pool(name="xg", bufs=xbufs) as xpool,
            tc.tile_pool(name="sq", bufs=sqbufs) as sqpool,
            tc.tile_pool(name="oh", bufs=3) as ohpool,
            tc.tile_pool(name="singles", bufs=1) as singles,
            tc.tile_pool(name="psum", bufs=1, space="PSUM") as psum,
        ):
            # Persistent tiles
            # t goes via the gpsimd (SWDGE) queue so the sync HWDGE queue's
            # first dispatch is already the group-0 x stream.
            t_all_i = singles.tile([P, qmain], i32)
            nc.gpsimd.dma_start(out=t_all_i[:], in_=t_main)
            t_all = singles.tile([P, qmain], f32)
            nc.vector.tensor_copy(t_all[:], t_all_i[:])
            iota10_i = singles.tile([P, C], i32)
            nc.gpsimd.iota(iota10_i[:], pattern=[[1, C]], base=0, channel_multiplier=0)
            iota10 = singles.tile([P, C], f32)
            nc.vector.tensor_copy(iota10[:], iota10_i[:])

            acc = singles.tile([P, g, C], f32)  # per-partition one-hot sums
            nc.vector.memset(acc[:], 0.0)

            p_sums = psum.tile([C, D], f32)
            p_ssq = psum.tile([C, D], f32)

            first = True
            for i0, gl in groups:
                xg = xpool.tile([P, gl, D], f32, tag="xg")
                nc.sync.dma_start(out=xg[:], in_=x_main[:, i0 : i0 + gl, :])

                xb = sqpool.tile([P, gl, D], f16, tag="xb")
                nc.scalar.copy(xb[:], xg[:])
                sqg = sqpool.tile([P, gl, D], f16, tag="sqg")
                nc.vector.tensor_tensor(
                    out=sqg[:], in0=xb[:], in1=xb[:], op=mybir.AluOpType.mult
                )

                ogb = ohpool.tile([P, gl, C], f16, tag="ogb")
                nc.vector.tensor_tensor(
                    out=ogb[:],
                    in0=t_all[:, i0 : i0 + gl, None].to_broadcast([P, gl, C]),
                    in1=iota10[:, None, :].to_broadcast([P, gl, C]),
                    op=eq,
                )
                nc.vector.tensor_tensor(
                    out=acc[:, 0:gl, :], in0=acc[:, 0:gl, :], in1=ogb[:], op=add
                )

                for k in range(gl):
                    nc.tensor.matmul(
                        out=p_sums[:],
                        lhsT=ogb[:, k, :],
                        rhs=xb[:, k, :],
                        start=first,
                        stop=False,
                    )
                    nc.tensor.matmul(
                        out=p_ssq[:],
                        lhsT=ogb[:, k, :],
                        rhs=sqg[:, k, :],
                        start=first,
                        stop=False,
                    )
                    first = False

            # Ragged tail: `tail` leftover rows go into partitions [0, tail) of
            # one extra subtile; unused partitions are zeroed so they add 0.
            xt = singles.tile([P, D], f32)
            nc.vector.memset(xt[:], 0.0)
            otb = singles.tile([P, C], f16)
            nc.vector.memset(otb[:], 0.0)
            if tail:
                tt_i = singles.tile([P, 1], i32)
                tt = singles.tile([P, 1], f32)
                nc.sync.dma_start(out=xt[0:tail, :], in_=x_d.ap()[qmain * P : ns, :])
                nc.sync.dma_start(
                    out=tt_i[0:tail, :], in_=t_d.ap()[qmain * P : ns, None]
                )
                nc.vector.tensor_copy(tt[0:tail, :], tt_i[0:tail, :])
                nc.vector.tensor_tensor(
                    out=otb[0:tail, :],
                    in0=tt[0:tail, 0:1].to_broadcast([tail, C]),
                    in1=iota10[0:tail, :],
                    op=eq,
                )
            xbt = singles.tile([P, D], f16)
            nc.scalar.copy(xbt[:], xt[:])
            sqt = singles.tile([P, D], f16)
            nc.vector.tensor_tensor(
                out=sqt[:], in0=xbt[:], in1=xbt[:], op=mybir.AluOpType.mult
            )
            nc.vector.tensor_tensor(
                out=acc[:, 0, :], in0=acc[:, 0, :], in1=otb[:], op=add
            )

            nc.tensor.matmul(
                out=p_sums[:], lhsT=otb[:], rhs=xbt[:], start=first, stop=True
            )
            nc.tensor.matmul(
                out=p_ssq[:], lhsT=otb[:], rhs=sqt[:], start=first, stop=True
            )

            # counts: reduce acc over the g axis -> [P, C]; host sums partitions
            cnt128 = singles.tile([P, C], f32)
            nc.vector.tensor_reduce(
                out=cnt128[:],
                in_=acc[:].rearrange("p g c -> p c g"),
                axis=mybir.AxisListType.X,
                op=add,
            )
            nc.sync.dma_start(out=cnt_d.ap()[:], in_=cnt128[:])

            out_sb = singles.tile([C, 2 * D], f32)
            nc.scalar.copy(out_sb[:, 0:D], p_sums[:])
            nc.scalar.copy(out_sb[:, D : 2 * D], p_ssq[:])
            nc.sync.dma_start(out=out_d.ap()[:], in_=out_sb[:])

    nc.compile()
    return nc, "out", "cnt"


def _get_program(ns, g):
    key = (ns, g)
    if key not in _CACHE:
        _CACHE[key] = _build(ns, g)
    return _CACHE[key]


def _finalize(partials, cnts):
    """partials: [ncores, C, 2D]; cnts: [ncores, P, C] -> final [1] fp32."""
    acc = partials.astype(np.float64).sum(axis=0)
    sums = acc[:, 0:D]
    ssq = acc[:, D : 2 * D]
    cnt = cnts.astype(np.float64).sum(axis=(0, 1))
    s2 = ssq.sum(axis=1)
    corr = (sums * sums).sum(axis=1) / cnt
    trace_per_class = (s2 - corr) / (cnt - 1.0)
    result = trace_per_class.sum() / NUM_CLASSES
    return np.asarray([result], dtype=np.float32)


def kernel(x, t):
    from concourse.bass_utils import run_bass_kernel_spmd

    x = np.ascontiguousarray(np.asarray(x, dtype=np.float32))
    t = np.ascontiguousarray(np.asarray(t, dtype=np.int32))
    assert x.shape == (N, D) and t.shape == (N,), (x.shape, t.shape)

    nc, out_name, cnt_name = _get_program(NSHARD, G)
    in_maps = [
        {
            "x": x[k * NSHARD : (k + 1) * NSHARD],
            "t": t[k * NSHARD : (k + 1) * NSHARD],
        }
        for k in range(NCORES)
    ]
    res = run_bass_kernel_spmd(nc, in_maps, core_ids=list(range(NCORES)))
    partials = np.stack([res.results[k][out_name] for k in range(NCORES)])
    cnts = np.stack([res.results[k][cnt_name] for k in range(NCORES)])
    return _finalize(partials, cnts)

